# revision 3
# baseline (speedup 1.0000x reference)
"""Trainium2 Bass kernel for ComplexSpatialAttentionModule.

Module: x:[4,256,64,64] f32 -> 1x1-conv q/k/v spatial attention (N=4096 tokens,
C=256 channels, C/8=32 qk dims) -> 1x1-conv out proj -> +residual.

Sharding: 8 cores = 4 batches x 2 query-halves. Each core holds the full
image of its batch (for K and V over all 4096 keys) and computes attention
rows for its 2048 query tokens. SPMD: one Bass program, per-core input maps.

Math restructuring (vs the naive reference):
  - softmax without max-subtraction: logits = q.k with |logit| <~ 20 for this
    data distribution, exp() is fp32-safe unshifted.
  - denominator sum_n exp(s[n,m]) via an all-ones stationary matmul operand:
    gives the per-column sum replicated over all 128 partitions (PSUM fp32,
    exact), which doubles as the partition-broadcast needed for the divide.
  - v-bias commutes out of attention (sum_n attn = 1) and out-projection is
    linear, so bias is applied once at the end as bo2 = wo@bv + bo.
  - normalization (divide by denominator, a per-query scalar) commutes with
    the out-projection contraction over channels; applied to the [256,m]
    attention output before wo (cheap) instead of the [4096,m] weights.

Layouts (partition dim first):
  xbf   [128, 2, 4096] bf16   channels (c = t*128+p) x keys  (lhsT for vT)
  k     [32, 4096]     bf16   qk-dim x keys   (lhsT of logits^T matmul)
  q     [32, 2048]     bf16   qk-dim x queries (rhs of logits^T matmul)
  vT    [128, 32, 256] bf16   keys (n = t_n*128+p) x channels (lhsT of attn@v)
  aT    [128, 512]     bf16   exp(logits^T): keys x queries (rhs of attn@v)
"""

import numpy as np
import ml_dtypes

import concourse.bacc as bacc
import concourse.mybir as mybir
import concourse.tile as tile
from concourse.bass import ts
from concourse.bass_utils import run_bass_kernel_spmd

F32 = mybir.dt.float32
BF16 = mybir.dt.bfloat16
AF = mybir.ActivationFunctionType
BF16_NP = ml_dtypes.bfloat16

C = 256      # channels
D = 32       # q/k dim (C/8)
N = 4096     # key tokens per batch
M = 2048     # query tokens per core (N/2)
MCH = 512    # query chunk (one PSUM bank of fp32)
NT = 128     # key tile (matmul contraction dim)
N_CORES = 8

LAST_RESULTS = None  # BassKernelResults of the most recent run (for test.py)
LAST_IN_MAPS = None  # per-core input maps of the most recent run (for test.py)
_NC_CACHE = None


def build_nc():
    nc = bacc.Bacc("TRN2", target_bir_lowering=False, debug=False)

    # Per-core inputs. All pre-laid-out on host so every DMA is a plain copy.
    xbf_d = nc.dram_tensor("xbf", [128, 2, N], BF16, kind="ExternalInput")
    xqbf_d = nc.dram_tensor("xqbf", [128, 2, M], BF16, kind="ExternalInput")
    xq32_d = nc.dram_tensor("xq32", [128, 2, M], F32, kind="ExternalInput")
    wqT_d = nc.dram_tensor("wqT", [128, 2, D], BF16, kind="ExternalInput")
    wkT_d = nc.dram_tensor("wkT", [128, 2, D], BF16, kind="ExternalInput")
    wvT_d = nc.dram_tensor("wvT", [128, 2, C], BF16, kind="ExternalInput")
    woT_d = nc.dram_tensor("woT", [128, 2, C], BF16, kind="ExternalInput")
    bq_d = nc.dram_tensor("bq", [D, 1], F32, kind="ExternalInput")
    bk_d = nc.dram_tensor("bk", [D, 1], F32, kind="ExternalInput")
    bo2_d = nc.dram_tensor("bo2", [128, 2], F32, kind="ExternalInput")
    out_d = nc.dram_tensor("out", [128, 2, M], F32, kind="ExternalOutput")

    with tile.TileContext(nc) as tc:
        with (
            tc.tile_pool(name="consts", bufs=1) as consts,
            tc.tile_pool(name="work", bufs=4) as work,
            tc.tile_pool(name="psum", bufs=2, space="PSUM") as psum,
        ):
            # ---- constants / inputs into SBUF ----
            ones_sb = consts.tile([128, NT], BF16)
            nc.vector.memset(ones_sb, 1.0)

            wqT_sb = consts.tile([128, 2, D], BF16)
            nc.sync.dma_start(out=wqT_sb, in_=wqT_d[:, :, :])
            wkT_sb = consts.tile([128, 2, D], BF16)
            nc.sync.dma_start(out=wkT_sb, in_=wkT_d[:, :, :])
            wvT_sb = consts.tile([128, 2, C], BF16)
            nc.sync.dma_start(out=wvT_sb, in_=wvT_d[:, :, :])
            woT_sb = consts.tile([128, 2, C], BF16)
            nc.sync.dma_start(out=woT_sb, in_=woT_d[:, :, :])
            bq_sb = consts.tile([D, 1], F32)
            nc.sync.dma_start(out=bq_sb, in_=bq_d[:, :])
            bk_sb = consts.tile([D, 1], F32)
            nc.sync.dma_start(out=bk_sb, in_=bk_d[:, :])
            bo2_sb = consts.tile([128, 2], F32)
            nc.sync.dma_start(out=bo2_sb, in_=bo2_d[:, :])

            # x in bf16, chunked so downstream matmuls can start early
            xbf_sb = consts.tile([128, 2, N], BF16)
            for i in range(4):
                nc.sync.dma_start(
                    out=xbf_sb[:, :, ts(i, N // 4)], in_=xbf_d[:, :, ts(i, N // 4)]
                )
            xqbf_sb = consts.tile([128, 2, M], BF16)
            for i in range(2):
                nc.sync.dma_start(
                    out=xqbf_sb[:, :, ts(i, M // 2)], in_=xqbf_d[:, :, ts(i, M // 2)]
                )
            xq32_sb = consts.tile([128, 2, M], F32)
            for i in range(2):
                nc.sync.dma_start(
                    out=xq32_sb[:, :, ts(i, M // 2)], in_=xq32_d[:, :, ts(i, M // 2)]
                )

            # ---- projections ----
            # q[d, m] = sum_c wq[d,c] x[c,m]  (+bq via ACT bias on psum read)
            q_sb = consts.tile([D, M], BF16)
            for j in range(M // MCH):
                pq = psum.tile([D, MCH], F32, tag="ps")
                for t in range(2):
                    nc.tensor.matmul(
                        pq,
                        wqT_sb[:, t, :],
                        xqbf_sb[:, t, ts(j, MCH)],
                        start=(t == 0),
                        stop=(t == 1),
                    )
                nc.scalar.activation(
                    out=q_sb[:, ts(j, MCH)], in_=pq, func=AF.Identity, bias=bq_sb
                )

            # k[d, n] over the full key range
            k_sb = consts.tile([D, N], BF16)
            for j in range(N // MCH):
                pk = psum.tile([D, MCH], F32, tag="ps")
                for t in range(2):
                    nc.tensor.matmul(
                        pk,
                        wkT_sb[:, t, :],
                        xbf_sb[:, t, ts(j, MCH)],
                        start=(t == 0),
                        stop=(t == 1),
                    )
                nc.scalar.activation(
                    out=k_sb[:, ts(j, MCH)], in_=pk, func=AF.Identity, bias=bk_sb
                )

            # vT[n, c] = sum_ci x[ci, n] wvT[ci, c]   (no bias: folded into bo2)
            vT_sb = consts.tile([128, N // NT, C], BF16)
            for t in range(N // NT):
                pv = psum.tile([128, C], F32, tag="po")
                for kk in range(2):
                    nc.tensor.matmul(
                        pv,
                        xbf_sb[:, kk, ts(t, NT)],
                        wvT_sb[:, kk, :],
                        start=(kk == 0),
                        stop=(kk == 1),
                    )
                nc.scalar.copy(out=vT_sb[:, t, :], in_=pv)

            # ---- attention main loop ----
            for j in range(M // MCH):
                po0 = psum.tile([128, MCH], F32, tag="po")
                po1 = psum.tile([128, MCH], F32, tag="po")
                pd = psum.tile([128, MCH], F32, tag="pd", bufs=1)
                for t in range(N // NT):
                    ps = psum.tile([128, MCH], F32, tag="ps")
                    # logits^T[n, m] for this (key-tile, query-chunk)
                    nc.tensor.matmul(
                        ps, k_sb[:, ts(t, NT)], q_sb[:, ts(j, MCH)], start=True, stop=True
                    )
                    aT = work.tile([128, MCH], BF16, tag="aT", bufs=6)
                    nc.scalar.activation(out=aT, in_=ps, func=AF.Exp)
                    first, last = t == 0, t == N // NT - 1
                    nc.tensor.matmul(
                        po0, vT_sb[:, t, 0:128], aT, start=first, stop=last
                    )
                    nc.tensor.matmul(
                        po1, vT_sb[:, t, 128:256], aT, start=first, stop=last
                    )
                    # softmax denominator, replicated across partitions
                    nc.tensor.matmul(pd, ones_sb, aT, start=first, stop=last)

                rd = work.tile([128, MCH], F32, tag="rd", bufs=2)
                nc.vector.reciprocal(rd, pd)
                # normalized attention output, bf16 for the wo matmul
                ub0 = work.tile([128, MCH], BF16, tag="ub", bufs=4)
                nc.vector.tensor_mul(ub0, po0, rd)
                ub1 = work.tile([128, MCH], BF16, tag="ub", bufs=4)
                nc.vector.tensor_mul(ub1, po1, rd)

                # out[c, m] = sum_ci wo[c,ci] attn[ci,m]  (+bo2 +residual)
                for ci in range(2):
                    pf = psum.tile([128, MCH], F32, tag="pf")
                    nc.tensor.matmul(
                        pf, woT_sb[:, 0, ts(ci, 128)], ub0, start=True, stop=False
                    )
                    nc.tensor.matmul(
                        pf, woT_sb[:, 1, ts(ci, 128)], ub1, start=False, stop=True
                    )
                    osb = work.tile([128, MCH], F32, tag="osb", bufs=4)
                    nc.scalar.activation(
                        out=osb, in_=pf, func=AF.Identity, bias=bo2_sb[:, ci : ci + 1]
                    )
                    nc.vector.tensor_add(osb, osb, xq32_sb[:, ci, ts(j, MCH)])
                    nc.sync.dma_start(out=out_d[:, ci, ts(j, MCH)], in_=osb)

    nc.finalize()
    return nc


def _to_pdim(a2d, inner):
    """[256, inner] row-major -> [128, 2, inner] (partition, c-tile, free)."""
    return np.ascontiguousarray(a2d.reshape(2, 128, inner).transpose(1, 0, 2))


def kernel(x, wq, bq, wk, bk, wv, bv, wo, bo):
    global LAST_RESULTS
    x = np.asarray(x, dtype=np.float32)
    B, Cc, H, W = x.shape
    assert (B, Cc, H * W) == (4, C, N)
    xf = x.reshape(B, C, N)

    wq = np.asarray(wq, np.float32)
    wk = np.asarray(wk, np.float32)
    wv = np.asarray(wv, np.float32)
    wo = np.asarray(wo, np.float32)
    bq = np.asarray(bq, np.float32)
    bk = np.asarray(bk, np.float32)
    bv = np.asarray(bv, np.float32)
    bo = np.asarray(bo, np.float32)

    wqT = _to_pdim(wq.T.astype(BF16_NP), D)
    wkT = _to_pdim(wk.T.astype(BF16_NP), D)
    wvT = _to_pdim(wv.T.astype(BF16_NP), C)
    woT = _to_pdim(wo.T.astype(BF16_NP), C)
    bo2 = np.ascontiguousarray(
        (wo @ bv + bo).astype(np.float32).reshape(2, 128).T
    )
    bq_c = np.ascontiguousarray(bq.reshape(D, 1))
    bk_c = np.ascontiguousarray(bk.reshape(D, 1))

    xbf = xf.astype(BF16_NP)  # [4, 256, 4096]

    in_maps = []
    for core in range(N_CORES):
        b, half = divmod(core, 2)
        m0 = half * M
        in_maps.append(
            {
                "xbf": _to_pdim(xbf[b], N),
                "xqbf": _to_pdim(xbf[b][:, m0 : m0 + M], M),
                "xq32": _to_pdim(xf[b][:, m0 : m0 + M], M),
                "wqT": wqT,
                "wkT": wkT,
                "wvT": wvT,
                "woT": woT,
                "bq": bq_c,
                "bk": bk_c,
                "bo2": bo2,
            }
        )

    global _NC_CACHE, LAST_IN_MAPS
    if _NC_CACHE is None:
        _NC_CACHE = build_nc()
    LAST_IN_MAPS = in_maps
    res = run_bass_kernel_spmd(_NC_CACHE, in_maps, core_ids=list(range(N_CORES)))
    LAST_RESULTS = res

    out = np.empty((B, C, N), np.float32)
    for core in range(N_CORES):
        b, half = divmod(core, 2)
        o = res.results[core]["out"]  # [128, 2, M]
        out[b][:, half * M : (half + 1) * M] = o.transpose(1, 0, 2).reshape(C, M)
    return out.reshape(B, Cc, H, W)
